# revision 1
# baseline (speedup 1.0000x reference)
"""GQA attention kernel for Trainium2, 8 NeuronCores.

Problem: B=1, S=4096, HIDDEN=2048, 8 query heads x d=256, 1 shared KV head,
causal mask, fp32.

Sharding: head-parallel attention with seq-split projections.
Per core j (owning head h=j and row block rows[512j:512j+512]):
  1. load x_own [512, 2048], PE-transpose -> xT_own [2048, 512]
  2. kv proj (own rows) -> AllGather kv [4096,256] and kvT [2048,512 blocks]
  3. q proj (own rows, all heads) -> qT_own [2048, 512] -> AllToAll ->
     qT_h [256, 4096] (own head, all rows)
  4. causal flash attention (no max-subtraction; fp32 range is plenty):
     scores^T = kvT.T-slices @ qT, exp on ScalarE (scale=1/16 folded in),
     denominator via ones-vector matmul, AV accumulation in PSUM.
  5. normalize, AllToAll -> attnout^T all heads for own rows [2048, 512]
  6. output projection (own rows) + bo -> out [512, 2048]
Host concatenates the 8 row blocks.

Matmuls run as float32r (full-rate fp32 path on the PE).
"""

import sys

import numpy as np

sys.path.insert(0, "/opt/trn_rl_repo")

S = 4096
HID = 2048
NH = 8
D = 256
NCORES = 8
R = S // NCORES  # 512 rows per core
NEG = -1.0e9
SCALE = 1.0 / 16.0  # 1/sqrt(256)

_BUILT = None


def _build():
    global _BUILT
    if _BUILT is not None:
        return _BUILT

    from contextlib import ExitStack

    from concourse import bacc, bass, tile
    from concourse.bass import mybir

    dt = mybir.dt
    f32 = dt.float32
    f32r = dt.float32r
    AF = mybir.ActivationFunctionType

    nc = bacc.Bacc(
        "TRN2",
        target_bir_lowering=False,
        debug=False,
        num_devices=NCORES,
    )

    # ---- DRAM I/O ----
    x_own = nc.dram_tensor("x_own", [R, HID], f32, kind="ExternalInput")
    wq2d = nc.dram_tensor("wq2d", [HID, HID], f32, kind="ExternalInput")
    bq_col = nc.dram_tensor("bq_col", [HID, 1], f32, kind="ExternalInput")
    wkv2d = nc.dram_tensor("wkv2d", [HID, D], f32, kind="ExternalInput")
    bkv_col = nc.dram_tensor("bkv_col", [D, 1], f32, kind="ExternalInput")
    bkv_row = nc.dram_tensor("bkv_row", [1, D], f32, kind="ExternalInput")
    wo2d = nc.dram_tensor("wo2d", [HID, HID], f32, kind="ExternalInput")
    bo_row = nc.dram_tensor("bo_row", [1, HID], f32, kind="ExternalInput")
    out = nc.dram_tensor("out", [R, HID], f32, kind="ExternalOutput")

    # ---- internal DRAM (collective buffers) ----
    grp = [list(range(NCORES))]
    qT_send = nc.dram_tensor("qT_send", [HID, R], f32)
    qT_recv = nc.dram_tensor("qT_recv", [HID, R], f32)
    kvT_send = nc.dram_tensor("kvT_send", [D, R], f32)
    kvT_all = nc.dram_tensor("kvT_all", [NCORES * D, R], f32, addr_space="Shared")
    kv_send = nc.dram_tensor("kv_send", [R, D], f32)
    kv_all = nc.dram_tensor("kv_all", [S, D], f32, addr_space="Shared")
    ao_send = nc.dram_tensor("ao_send", [HID, R], f32)
    ao_recv = nc.dram_tensor("ao_recv", [HID, R], f32)

    # ---- compile-time constants (embedded in NEFF) ----
    ident_np = np.eye(128, dtype=np.float32)
    ones_col_np = np.ones((128, 1), dtype=np.float32)
    ones_row_np = np.ones((1, 128), dtype=np.float32)
    # diagonal masks for a 512-row q block vs its two 256-key diagonal groups
    # layout [128 keys, 2 groups * 2 slices * 512 rows]
    mask_np = np.empty((128, 2048), dtype=np.float32)
    kappa = np.arange(128)[:, None]
    rows = np.arange(512)[None, :]
    for grel in range(2):
        for sl in range(2):
            keyrel = 256 * grel + 128 * sl + kappa
            blk = np.where(keyrel <= rows, 0.0, NEG).astype(np.float32)
            mask_np[:, 1024 * grel + 512 * sl : 1024 * grel + 512 * sl + 512] = blk
    ident_d = nc.inline_tensor(ident_np, "ident")
    ones_col_d = nc.inline_tensor(ones_col_np, "ones_col")
    ones_row_d = nc.inline_tensor(ones_row_np, "ones_row")
    mask_d = nc.inline_tensor(mask_np, "mask_const")

    def r32(ap):
        return ap.bitcast(f32r)

    with tile.TileContext(nc) as tc:
        with ExitStack() as top:
            cpool = top.enter_context(tc.tile_pool(name="const", bufs=1))
            ident = cpool.tile([128, 128], f32, tag="ident")
            nc.sync.dma_start(ident[:], ident_d[:])
            ones_col = cpool.tile([128, 1], f32, tag="ones_col")
            nc.sync.dma_start(r32(ones_col[:]), r32(ones_col_d[:]))
            ones_row = cpool.tile([1, 128], f32, tag="ones_row")
            nc.sync.dma_start(r32(ones_row[:]), r32(ones_row_d[:]))
            bq_sb = cpool.tile([128, 16], f32, tag="bq")
            for gd in range(16):
                nc.sync.dma_start(
                    bq_sb[:, gd : gd + 1], bq_col[128 * gd : 128 * gd + 128, :]
                )
            bkv_sb = cpool.tile([128, 2], f32, tag="bkv")
            for dh in range(2):
                nc.sync.dma_start(
                    bkv_sb[:, dh : dh + 1], bkv_col[128 * dh : 128 * dh + 128, :]
                )
            bkvr_sb = cpool.tile([1, D], f32, tag="bkvr")
            nc.sync.dma_start(r32(bkvr_sb[:]), r32(bkv_row[:]))
            bor_sb = cpool.tile([1, HID], f32, tag="bor")
            nc.sync.dma_start(r32(bor_sb[:]), r32(bo_row[:]))

            # ============ phase 1: load x_own and transpose ============
            with ExitStack() as ph123:
                xT_pool = ph123.enter_context(tc.tile_pool(name="xT", bufs=1))
                xT = xT_pool.tile([128, 16 * R], f32, tag="xT")  # hs-slice layout
                with ExitStack() as ph1:
                    xraw_pool = ph1.enter_context(tc.tile_pool(name="xraw", bufs=2))
                    tp_psum = ph1.enter_context(
                        tc.tile_pool(name="tp_psum", bufs=2, space="PSUM")
                    )
                    for rc in range(4):
                        xr = xraw_pool.tile([128, HID], f32, tag="xr")
                        nc.sync.dma_start(xr[:], x_own[128 * rc : 128 * rc + 128, :])
                        for hs in range(16):
                            tp = tp_psum.tile([128, 128], f32, tag="tp")
                            nc.tensor.transpose(
                                tp[:], xr[:, 128 * hs : 128 * hs + 128], ident[:]
                            )
                            nc.vector.tensor_copy(
                                r32(xT[:, R * hs + 128 * rc : R * hs + 128 * rc + 128]),
                                tp[:],
                            )

                # ============ phase 2: kv projection + AllGather ============
                with ExitStack() as ph2:
                    wkv_pool = ph2.enter_context(tc.tile_pool(name="wkv", bufs=1))
                    wkv_sb = wkv_pool.tile([128, 16 * D], f32, tag="wkv")
                    for hs in range(16):
                        nc.sync.dma_start(
                            r32(wkv_sb[:, D * hs : D * hs + D]),
                            r32(wkv2d[128 * hs : 128 * hs + 128, :]),
                        )
                    kv_psum = ph2.enter_context(
                        tc.tile_pool(name="kv_psum", bufs=2, space="PSUM")
                    )
                    kv_out = ph2.enter_context(tc.tile_pool(name="kv_out", bufs=2))
                    # kvT_own [256, 512]
                    for dh in range(2):
                        ps = kv_psum.tile([128, R], f32, tag="kvps")
                        for hs in range(16):
                            nc.tensor.matmul(
                                ps[:],
                                r32(wkv_sb[:, D * hs + 128 * dh : D * hs + 128 * dh + 128]),
                                r32(xT[:, R * hs : R * hs + R]),
                                start=(hs == 0),
                                stop=(hs == 15),
                            )
                        kvt_sb = kv_out.tile([128, R], f32, tag="kvt")
                        nc.scalar.activation(
                            kvt_sb[:], ps[:], AF.Identity, bias=bkv_sb[:, dh : dh + 1]
                        )
                        nc.sync.dma_start(
                            kvT_send[128 * dh : 128 * dh + 128, :], kvt_sb[:]
                        )
                    # kv_own [512, 256]
                    for rc in range(4):
                        ps = kv_psum.tile([128, D], f32, tag="kvps")
                        for hs in range(16):
                            nc.tensor.matmul(
                                ps[:],
                                r32(xT[:, R * hs + 128 * rc : R * hs + 128 * rc + 128]),
                                r32(wkv_sb[:, D * hs : D * hs + D]),
                                start=(hs == 0),
                                stop=False,
                            )
                        nc.tensor.matmul(
                            ps[:],
                            r32(ones_row[:]),
                            r32(bkvr_sb[:]),
                            start=False,
                            stop=True,
                        )
                        kvr_sb = kv_out.tile([128, D], f32, tag="kvr")
                        nc.vector.tensor_copy(kvr_sb[:], ps[:])
                        nc.sync.dma_start(
                            kv_send[128 * rc : 128 * rc + 128, :], kvr_sb[:]
                        )
                    nc.gpsimd.collective_compute(
                        "AllGather",
                        mybir.AluOpType.bypass,
                        replica_groups=grp,
                        ins=[kvT_send[:]],
                        outs=[kvT_all[:]],
                    )
                    nc.gpsimd.collective_compute(
                        "AllGather",
                        mybir.AluOpType.bypass,
                        replica_groups=grp,
                        ins=[kv_send[:]],
                        outs=[kv_all[:]],
                    )

                # ============ phase 3: q projection + AllToAll ============
                with ExitStack() as ph3:
                    wq_pool = ph3.enter_context(tc.tile_pool(name="wq", bufs=4))
                    q_psum = ph3.enter_context(
                        tc.tile_pool(name="q_psum", bufs=4, space="PSUM")
                    )
                    q_out = ph3.enter_context(tc.tile_pool(name="q_out", bufs=4))
                    for gdg in range(4):
                        pss = [q_psum.tile([128, R], f32, tag="qps", name=f"qps{gdg}_{i}") for i in range(4)]
                        for hs in range(16):
                            wq_t = wq_pool.tile([128, 512], f32, tag="wqt")
                            nc.sync.dma_start(
                                r32(wq_t[:]),
                                r32(wq2d[
                                    128 * hs : 128 * hs + 128,
                                    512 * gdg : 512 * gdg + 512,
                                ]),
                            )
                            for sub in range(4):
                                nc.tensor.matmul(
                                    pss[sub][:],
                                    r32(wq_t[:, 128 * sub : 128 * sub + 128]),
                                    r32(xT[:, R * hs : R * hs + R]),
                                    start=(hs == 0),
                                    stop=(hs == 15),
                                )
                        for sub in range(4):
                            gd = 4 * gdg + sub
                            q_sb = q_out.tile([128, R], f32, tag="qsb")
                            nc.scalar.activation(
                                q_sb[:],
                                pss[sub][:],
                                AF.Identity,
                                bias=bq_sb[:, gd : gd + 1],
                            )
                            nc.sync.dma_start(
                                qT_send[128 * gd : 128 * gd + 128, :], q_sb[:]
                            )
                    nc.gpsimd.collective_compute(
                        "AllToAll",
                        mybir.AluOpType.bypass,
                        replica_groups=grp,
                        ins=[qT_send[:]],
                        outs=[qT_recv[:]],
                    )

            # ============ phase 4: load attention operands ============
            with ExitStack() as ph45:
                attn_pool = ph45.enter_context(tc.tile_pool(name="attn", bufs=1))
                qT = attn_pool.tile([128, 2 * S], f32, tag="qT")
                kvT = attn_pool.tile([128, 2 * S], f32, tag="kvT")
                kv_sb = attn_pool.tile([128, 32 * D], f32, tag="kv_sb")
                mask_sb = attn_pool.tile([128, 2048], f32, tag="mask")
                nc.sync.dma_start(mask_sb[:], mask_d[:])
                for src in range(8):
                    for dh in range(2):
                        nc.sync.dma_start(
                            r32(qT[:, S * dh + R * src : S * dh + R * src + R]),
                            r32(qT_recv[D * src + 128 * dh : D * src + 128 * dh + 128, :]),
                        )
                        nc.sync.dma_start(
                            r32(kvT[:, S * dh + R * src : S * dh + R * src + R]),
                            r32(kvT_all[D * src + 128 * dh : D * src + 128 * dh + 128, :]),
                        )
                for kt in range(32):
                    nc.sync.dma_start(
                        r32(kv_sb[:, D * kt : D * kt + D]),
                        r32(kv_all[128 * kt : 128 * kt + 128, :]),
                    )

                # ============ phase 5: causal flash attention ============
                s_psum = ph45.enter_context(
                    tc.tile_pool(name="s_psum", bufs=2, space="PSUM")
                )
                ao_psum = ph45.enter_context(
                    tc.tile_pool(name="ao_psum", bufs=2, space="PSUM")
                )
                den_psum = ph45.enter_context(
                    tc.tile_pool(name="den_psum", bufs=1, space="PSUM")
                )
                p_pool = ph45.enter_context(tc.tile_pool(name="p", bufs=3))
                nrm_pool = ph45.enter_context(tc.tile_pool(name="nrm", bufs=2))
                aon_pool = ph45.enter_context(tc.tile_pool(name="aon", bufs=4))

                for b in range(8):
                    aops = [ao_psum.tile([128, R], f32, tag="aops", name=f"aops{b}_{i}") for i in range(2)]
                    denp = den_psum.tile([1, R], f32, tag="denp")
                    ngroups = 2 * (b + 1)
                    for kg in range(ngroups):
                        st = s_psum.tile([128, 1024], f32, tag="st")
                        for sl in range(2):
                            k = 2 * kg + sl
                            for dh in range(2):
                                nc.tensor.matmul(
                                    st[:, 512 * sl : 512 * sl + 512],
                                    r32(kvT[:, S * dh + 128 * k : S * dh + 128 * k + 128]),
                                    r32(qT[:, S * dh + R * b : S * dh + R * b + R]),
                                    start=(dh == 0),
                                    stop=(dh == 1),
                                )
                        if kg >= 2 * b:
                            grel = kg - 2 * b
                            nc.vector.tensor_add(
                                st[:],
                                st[:],
                                mask_sb[:, 1024 * grel : 1024 * grel + 1024],
                            )
                        pt = p_pool.tile([128, 1024], f32, tag="pt")
                        nc.scalar.activation(r32(pt[:]), st[:], AF.Exp, scale=SCALE)
                        for sl in range(2):
                            k = 2 * kg + sl
                            nc.tensor.matmul(
                                denp[:],
                                r32(ones_col[:]),
                                r32(pt[:, 512 * sl : 512 * sl + 512]),
                                start=(kg == 0 and sl == 0),
                                stop=(kg == ngroups - 1 and sl == 1),
                            )
                            for dh in range(2):
                                nc.tensor.matmul(
                                    aops[dh][:],
                                    r32(kv_sb[:, D * k + 128 * dh : D * k + 128 * dh + 128]),
                                    r32(pt[:, 512 * sl : 512 * sl + 512]),
                                    start=(kg == 0 and sl == 0),
                                    stop=(kg == ngroups - 1 and sl == 1),
                                )
                    # normalize + send
                    den_sb = nrm_pool.tile([1, R], f32, tag="den_sb")
                    nc.vector.reciprocal(den_sb[:], denp[:])
                    bc = nrm_pool.tile([128, R], f32, tag="bc")
                    nc.gpsimd.partition_broadcast(bc[:], den_sb[:])
                    for dh in range(2):
                        aon = aon_pool.tile([128, R], f32, tag="aon")
                        nc.vector.tensor_mul(aon[:], aops[dh][:], bc[:])
                        nc.sync.dma_start(
                            ao_send[D * b + 128 * dh : D * b + 128 * dh + 128, :],
                            aon[:],
                        )
                nc.gpsimd.collective_compute(
                    "AllToAll",
                    mybir.AluOpType.bypass,
                    replica_groups=grp,
                    ins=[ao_send[:]],
                    outs=[ao_recv[:]],
                )

            # ============ phase 6: output projection ============
            with ExitStack() as ph6:
                o_in = ph6.enter_context(tc.tile_pool(name="o_in", bufs=1))
                aoT = o_in.tile([128, 16 * R], f32, tag="aoT")
                for k in range(16):
                    nc.sync.dma_start(
                        r32(aoT[:, R * k : R * k + R]),
                        r32(ao_recv[128 * k : 128 * k + 128, :]),
                    )
                wo_sb = o_in.tile([128, 16 * HID], f32, tag="wo_sb")
                for k in range(16):
                    nc.sync.dma_start(
                        r32(wo_sb[:, HID * k : HID * k + HID]),
                        r32(wo2d[128 * k : 128 * k + 128, :]),
                    )
                o_psum = ph6.enter_context(
                    tc.tile_pool(name="o_psum", bufs=4, space="PSUM")
                )
                o_out = ph6.enter_context(tc.tile_pool(name="o_out", bufs=2))
                for rc in range(4):
                    osb = o_out.tile([128, HID], f32, tag="osb")
                    for ncol in range(4):
                        ps = o_psum.tile([128, 512], f32, tag="ops")
                        for k in range(16):
                            nc.tensor.matmul(
                                ps[:],
                                r32(aoT[:, R * k + 128 * rc : R * k + 128 * rc + 128]),
                                r32(wo_sb[:, HID * k + 512 * ncol : HID * k + 512 * ncol + 512]),
                                start=(k == 0),
                                stop=False,
                            )
                        nc.tensor.matmul(
                            ps[:],
                            r32(ones_row[:]),
                            r32(bor_sb[:, 512 * ncol : 512 * ncol + 512]),
                            start=False,
                            stop=True,
                        )
                        nc.vector.tensor_copy(
                            osb[:, 512 * ncol : 512 * ncol + 512], ps[:]
                        )
                    nc.sync.dma_start(out[128 * rc : 128 * rc + 128, :], osb[:])

    nc.compile()
    _BUILT = nc
    return nc


def _make_in_maps(x, wq, bq, wkv, bkv, wo, bo):
    x = np.asarray(x, dtype=np.float32).reshape(S, HID)
    shared = {
        "wq2d": np.ascontiguousarray(
            np.asarray(wq, dtype=np.float32).reshape(HID, HID)
        ),
        "bq_col": np.ascontiguousarray(
            np.asarray(bq, dtype=np.float32).reshape(HID, 1)
        ),
        "wkv2d": np.ascontiguousarray(
            np.asarray(wkv, dtype=np.float32).reshape(HID, D)
        ),
        "bkv_col": np.ascontiguousarray(
            np.asarray(bkv, dtype=np.float32).reshape(D, 1)
        ),
        "bkv_row": np.ascontiguousarray(
            np.asarray(bkv, dtype=np.float32).reshape(1, D)
        ),
        "wo2d": np.ascontiguousarray(
            np.asarray(wo, dtype=np.float32).reshape(HID, HID)
        ),
        "bo_row": np.ascontiguousarray(
            np.asarray(bo, dtype=np.float32).reshape(1, HID)
        ),
    }
    in_maps = []
    for j in range(NCORES):
        m = dict(shared)
        m["x_own"] = np.ascontiguousarray(x[R * j : R * j + R, :])
        in_maps.append(m)
    return in_maps


def _run(inputs, trace=False, **trace_kwargs):
    from concourse.bass_utils import run_bass_kernel_spmd

    nc = _build()
    in_maps = _make_in_maps(
        inputs["x"],
        inputs["wq"],
        inputs["bq"],
        inputs["wkv"],
        inputs["bkv"],
        inputs["wo"],
        inputs["bo"],
    )
    res = run_bass_kernel_spmd(
        nc, in_maps, list(range(NCORES)), trace=trace, **trace_kwargs
    )
    outs = [np.asarray(res.results[j]["out"]) for j in range(NCORES)]
    full = np.concatenate(outs, axis=0).reshape(1, S, HID).astype(np.float32)
    return full, res


def kernel(**inputs):
    full, _ = _run(inputs, trace=False)
    return full


if __name__ == "__main__":
    rng = np.random.default_rng(0)
    ins = {
        "x": rng.standard_normal((1, S, HID), dtype=np.float32),
        "wq": rng.standard_normal((HID, NH, D), dtype=np.float32) / 45.25,
        "bq": np.zeros((NH, D), np.float32),
        "wkv": rng.standard_normal((HID, 1, D), dtype=np.float32) / 45.25,
        "bkv": np.zeros((1, D), np.float32),
        "wo": rng.standard_normal((NH, D, HID), dtype=np.float32) / 45.25,
        "bo": np.zeros((HID,), np.float32),
        "mask": np.tril(np.ones((S, S), bool))[None, None],
    }
    out = kernel(**ins)
    print("out", out.shape, out.dtype, float(np.abs(out).max()))



# revision 2
# speedup vs baseline: 1.3162x; 1.3162x over previous
"""GQA attention kernel for Trainium2, 8 NeuronCores — v2.

Problem: B=1, S=4096, HIDDEN=2048, 8 query heads x d=256, 1 shared KV head,
causal mask, fp32 in/out.

Sharding v2: full tensor-parallel over heads with a replicated input.
Host-side, x is transposed (layout choice) and replicated to all cores in
bf16; core j owns head j end-to-end:
  1. stream xT chunks; project q (own head only) and kv (replicated —
     cheaper than the AllGather it replaces) in bf16; PE-transpose kv into
     rows-major for the AV stationary.
  2. causal flash attention for head j over all 4096 rows (no max
     subtraction; fp32 PSUM accumulation), diagonal blocks narrowed to the
     causal range.
  3. normalize, one bf16 AllToAll (head-major -> row-block-major).
  4. output projection of own 512-row block against full wo + bo.
Host concatenates the 8 row blocks.

This removes v1's two kv AllGathers and the q AllToAll (~170us of serial
collective time) at the cost of ~40us of replicated kv-projection compute.
bf16 matmul operands run at the same PE rate as fp32r but halve DMA/SBUF.
"""

import sys

import numpy as np

sys.path.insert(0, "/opt/trn_rl_repo")

S = 4096
HID = 2048
NH = 8
D = 256
NCORES = 8
R = 512  # output rows per core / q rows per attention block
CH = 512  # projection chunk rows
NCH = S // CH
NEG = -1.0e9
SCALE = 1.0 / 16.0  # 1/sqrt(256)

_BUILT = None


def _build():
    global _BUILT
    if _BUILT is not None:
        return _BUILT

    from contextlib import ExitStack

    import ml_dtypes

    from concourse import bacc, tile
    from concourse.bass import mybir

    dt = mybir.dt
    f32 = dt.float32
    bf16 = dt.bfloat16
    bfnp = ml_dtypes.bfloat16
    AF = mybir.ActivationFunctionType

    nc = bacc.Bacc(
        "TRN2",
        target_bir_lowering=False,
        debug=False,
        num_devices=NCORES,
    )

    # ---- DRAM I/O ----
    xT_d = nc.dram_tensor("xT", [HID, S], bf16, kind="ExternalInput")
    wq_d = nc.dram_tensor("wq_h", [HID, D], bf16, kind="ExternalInput")
    bq_col = nc.dram_tensor("bq_col", [D, 1], f32, kind="ExternalInput")
    wkv_d = nc.dram_tensor("wkv2d", [HID, D], bf16, kind="ExternalInput")
    bkv_col = nc.dram_tensor("bkv_col", [D, 1], f32, kind="ExternalInput")
    wo_d = nc.dram_tensor("wo2d", [HID, HID], bf16, kind="ExternalInput")
    bo_row = nc.dram_tensor("bo_row", [1, HID], f32, kind="ExternalInput")
    out = nc.dram_tensor("out", [R, HID], f32, kind="ExternalOutput")

    # ---- collective buffers ----
    grp = [list(range(NCORES))]
    ao_send = nc.dram_tensor("ao_send", [NH * D, R], bf16)
    ao_recv = nc.dram_tensor("ao_recv", [NH * D, R], bf16)

    # ---- compile-time constants ----
    ident_np = np.eye(128).astype(bfnp)
    ones_col_np = np.ones((128, 1)).astype(bfnp)
    # diagonal masks for a 512-row q block vs its two 256-key diagonal groups
    # layout [128 keys, 2 groups * 2 slices * 512 rows]
    mask_np = np.empty((128, 2048), dtype=np.float32)
    kappa = np.arange(128)[:, None]
    rows = np.arange(512)[None, :]
    for grel in range(2):
        for sl in range(2):
            keyrel = 256 * grel + 128 * sl + kappa
            blk = np.where(keyrel <= rows, 0.0, NEG).astype(np.float32)
            mask_np[:, 1024 * grel + 512 * sl : 1024 * grel + 512 * sl + 512] = blk
    ident_d = nc.inline_tensor(ident_np, "ident")
    ones_col_d = nc.inline_tensor(ones_col_np, "ones_col")
    mask_d = nc.inline_tensor(mask_np, "mask_const")

    with tile.TileContext(nc) as tc:
        with ExitStack() as top:
            cpool = top.enter_context(tc.tile_pool(name="const", bufs=1))
            ident = cpool.tile([128, 128], bf16, tag="ident")
            nc.sync.dma_start(ident[:], ident_d[:])
            ones_col = cpool.tile([128, 1], bf16, tag="ones_col")
            nc.sync.dma_start(ones_col[:], ones_col_d[:])
            bq_sb = cpool.tile([128, 2], f32, tag="bq")
            bkv_sb = cpool.tile([128, 2], f32, tag="bkv")
            for dh in range(2):
                nc.sync.dma_start(
                    bq_sb[:, dh : dh + 1], bq_col[128 * dh : 128 * dh + 128, :]
                )
                nc.sync.dma_start(
                    bkv_sb[:, dh : dh + 1], bkv_col[128 * dh : 128 * dh + 128, :]
                )
            bor_sb = cpool.tile([1, HID], f32, tag="bor")
            nc.sync.dma_start(bor_sb[:], bo_row[:])
            bo_bc = cpool.tile([128, HID], f32, tag="bo_bc")
            nc.gpsimd.partition_broadcast(bo_bc[:], bor_sb[:])

            # pools alive through projections + attention
            big = top.enter_context(tc.tile_pool(name="big", bufs=1))
            qT = big.tile([128, 2 * S], bf16, tag="qT")  # [d-slice, rows]
            kvT = big.tile([128, 2 * S], bf16, tag="kvT")  # [d-slice, keys]
            kv_sb = big.tile([128, 32 * D], bf16, tag="kv")  # rows-major kv
            mask_sb = big.tile([128, 2048], f32, tag="mask")
            nc.sync.dma_start(mask_sb[:], mask_d[:])

            wo_pool = top.enter_context(tc.tile_pool(name="wo", bufs=1))

            # ============ phase 1: projections (streamed over chunks) ========
            with ExitStack() as ph1:
                wpool = ph1.enter_context(tc.tile_pool(name="w", bufs=1))
                wq_sb = wpool.tile([128, 16 * D], bf16, tag="wq")
                wkv_sb = wpool.tile([128, 16 * D], bf16, tag="wkv")
                for hs in range(16):
                    nc.sync.dma_start(
                        wq_sb[:, D * hs : D * hs + D],
                        wq_d[128 * hs : 128 * hs + 128, :],
                    )
                    nc.sync.dma_start(
                        wkv_sb[:, D * hs : D * hs + D],
                        wkv_d[128 * hs : 128 * hs + 128, :],
                    )
                xr_pool = ph1.enter_context(tc.tile_pool(name="xr", bufs=2))
                pj_psum = ph1.enter_context(
                    tc.tile_pool(name="pj_psum", bufs=4, space="PSUM")
                )
                tp_psum = ph1.enter_context(
                    tc.tile_pool(name="tp_psum", bufs=2, space="PSUM")
                )
                for c in range(NCH):
                    xr = xr_pool.tile([128, 16 * CH], bf16, tag="xr")
                    for hs in range(16):
                        nc.sync.dma_start(
                            xr[:, CH * hs : CH * hs + CH],
                            xT_d[128 * hs : 128 * hs + 128, CH * c : CH * c + CH],
                        )
                    for dh in range(2):
                        qp = pj_psum.tile([128, CH], f32, tag="pj")
                        for hs in range(16):
                            nc.tensor.matmul(
                                qp[:],
                                wq_sb[:, D * hs + 128 * dh : D * hs + 128 * dh + 128],
                                xr[:, CH * hs : CH * hs + CH],
                                start=(hs == 0),
                                stop=(hs == 15),
                            )
                        nc.scalar.activation(
                            qT[:, S * dh + CH * c : S * dh + CH * c + CH],
                            qp[:],
                            AF.Identity,
                            bias=bq_sb[:, dh : dh + 1],
                        )
                        kp = pj_psum.tile([128, CH], f32, tag="pj")
                        for hs in range(16):
                            nc.tensor.matmul(
                                kp[:],
                                wkv_sb[:, D * hs + 128 * dh : D * hs + 128 * dh + 128],
                                xr[:, CH * hs : CH * hs + CH],
                                start=(hs == 0),
                                stop=(hs == 15),
                            )
                        nc.scalar.activation(
                            kvT[:, S * dh + CH * c : S * dh + CH * c + CH],
                            kp[:],
                            AF.Identity,
                            bias=bkv_sb[:, dh : dh + 1],
                        )
                    # rows-major kv via PE transpose (bf16)
                    for i4 in range(CH // 128):
                        kt = (CH // 128) * c + i4
                        for dh in range(2):
                            tp = tp_psum.tile([128, 128], bf16, tag="tp")
                            nc.tensor.transpose(
                                tp[:],
                                kvT[
                                    :,
                                    S * dh + CH * c + 128 * i4 : S * dh
                                    + CH * c
                                    + 128 * i4
                                    + 128,
                                ],
                                ident[:],
                            )
                            nc.vector.tensor_copy(
                                kv_sb[:, D * kt + 128 * dh : D * kt + 128 * dh + 128],
                                tp[:],
                            )

            # prefetch wo during attention
            wo_sb = wo_pool.tile([128, 16 * HID], bf16, tag="wo")
            for k in range(16):
                nc.sync.dma_start(
                    wo_sb[:, HID * k : HID * k + HID],
                    wo_d[128 * k : 128 * k + 128, :],
                )

            # ============ phase 2: causal flash attention ============
            with ExitStack() as ph2:
                s_psum = ph2.enter_context(
                    tc.tile_pool(name="s_psum", bufs=2, space="PSUM")
                )
                ao_psum = ph2.enter_context(
                    tc.tile_pool(name="ao_psum", bufs=2, space="PSUM")
                )
                den_psum = ph2.enter_context(
                    tc.tile_pool(name="den_psum", bufs=1, space="PSUM")
                )
                p_pool = ph2.enter_context(tc.tile_pool(name="p", bufs=3))
                nrm_pool = ph2.enter_context(tc.tile_pool(name="nrm", bufs=2))
                aon_pool = ph2.enter_context(tc.tile_pool(name="aon", bufs=4))

                for b in range(8):
                    aops = [
                        ao_psum.tile([128, R], f32, tag="aops", name=f"aops{b}_{i}")
                        for i in range(2)
                    ]
                    denp = den_psum.tile([1, R], f32, tag="denp")
                    ngroups = 2 * (b + 1)
                    for kg in range(ngroups):
                        diag = kg >= 2 * b
                        grel = kg - 2 * b
                        st = s_psum.tile([128, 1024], f32, tag="st")
                        rel = [
                            256 * grel + 128 * sl if diag else 0 for sl in range(2)
                        ]
                        for sl in range(2):
                            k = 2 * kg + sl
                            r0 = rel[sl]
                            for dh in range(2):
                                nc.tensor.matmul(
                                    st[:, 512 * sl + r0 : 512 * sl + 512],
                                    kvT[:, S * dh + 128 * k : S * dh + 128 * k + 128],
                                    qT[:, S * dh + R * b + r0 : S * dh + R * b + R],
                                    start=(dh == 0),
                                    stop=(dh == 1),
                                )
                        if diag:
                            for sl in range(2):
                                r0 = rel[sl]
                                nc.vector.tensor_add(
                                    st[:, 512 * sl + r0 : 512 * sl + 512],
                                    st[:, 512 * sl + r0 : 512 * sl + 512],
                                    mask_sb[
                                        :,
                                        1024 * grel + 512 * sl + r0 : 1024 * grel
                                        + 512 * sl
                                        + 512,
                                    ],
                                )
                        pt = p_pool.tile([128, 1024], bf16, tag="pt")
                        for sl in range(2):
                            r0 = rel[sl]
                            nc.scalar.activation(
                                pt[:, 512 * sl + r0 : 512 * sl + 512],
                                st[:, 512 * sl + r0 : 512 * sl + 512],
                                AF.Exp,
                                scale=SCALE,
                            )
                        for sl in range(2):
                            k = 2 * kg + sl
                            r0 = rel[sl]
                            first = kg == 0 and sl == 0
                            last = kg == ngroups - 1 and sl == 1
                            nc.tensor.matmul(
                                denp[:, r0:R],
                                ones_col[:],
                                pt[:, 512 * sl + r0 : 512 * sl + 512],
                                start=first,
                                stop=last,
                            )
                            for dh in range(2):
                                nc.tensor.matmul(
                                    aops[dh][:, r0:R],
                                    kv_sb[:, D * k + 128 * dh : D * k + 128 * dh + 128],
                                    pt[:, 512 * sl + r0 : 512 * sl + 512],
                                    start=first,
                                    stop=last,
                                )
                    # normalize + send (bf16)
                    den_sb = nrm_pool.tile([1, R], f32, tag="den_sb")
                    nc.vector.reciprocal(den_sb[:], denp[:])
                    bc = nrm_pool.tile([128, R], f32, tag="bc")
                    nc.gpsimd.partition_broadcast(bc[:], den_sb[:])
                    for dh in range(2):
                        aon = aon_pool.tile([128, R], bf16, tag="aon")
                        nc.vector.tensor_mul(aon[:], aops[dh][:], bc[:])
                        nc.sync.dma_start(
                            ao_send[D * b + 128 * dh : D * b + 128 * dh + 128, :],
                            aon[:],
                        )
                nc.gpsimd.collective_compute(
                    "AllToAll",
                    mybir.AluOpType.bypass,
                    replica_groups=grp,
                    ins=[ao_send[:]],
                    outs=[ao_recv[:]],
                )

            # ============ phase 3: output projection ============
            with ExitStack() as ph3:
                o_in = ph3.enter_context(tc.tile_pool(name="o_in", bufs=1))
                aoT = o_in.tile([128, 16 * R], bf16, tag="aoT")
                for k in range(16):
                    nc.sync.dma_start(
                        aoT[:, R * k : R * k + R],
                        ao_recv[128 * k : 128 * k + 128, :],
                    )
                o_psum = ph3.enter_context(
                    tc.tile_pool(name="o_psum", bufs=4, space="PSUM")
                )
                o_out = ph3.enter_context(tc.tile_pool(name="o_out", bufs=4))
                for rc in range(4):
                    for ncol in range(4):
                        ps = o_psum.tile([128, 512], f32, tag="ops")
                        for k in range(16):
                            nc.tensor.matmul(
                                ps[:],
                                aoT[:, R * k + 128 * rc : R * k + 128 * rc + 128],
                                wo_sb[
                                    :, HID * k + 512 * ncol : HID * k + 512 * ncol + 512
                                ],
                                start=(k == 0),
                                stop=(k == 15),
                            )
                        osb = o_out.tile([128, 512], f32, tag="osb")
                        nc.vector.tensor_add(
                            osb[:], ps[:], bo_bc[:, 512 * ncol : 512 * ncol + 512]
                        )
                        nc.sync.dma_start(
                            out[128 * rc : 128 * rc + 128, 512 * ncol : 512 * ncol + 512],
                            osb[:],
                        )

    nc.compile()
    _BUILT = nc
    return nc


def _make_in_maps(x, wq, bq, wkv, bkv, wo, bo):
    import ml_dtypes

    bfnp = ml_dtypes.bfloat16
    x2d = np.asarray(x, dtype=np.float32).reshape(S, HID)
    xT = x2d.T.astype(bfnp)  # [HID, S], contiguous
    wq3 = np.asarray(wq, dtype=np.float32).reshape(HID, NH, D)
    bq2 = np.asarray(bq, dtype=np.float32).reshape(NH, D)
    shared = {
        "xT": xT,
        "wkv2d": np.asarray(wkv, dtype=np.float32).reshape(HID, D).astype(bfnp),
        "bkv_col": np.ascontiguousarray(
            np.asarray(bkv, dtype=np.float32).reshape(D, 1)
        ),
        "wo2d": np.asarray(wo, dtype=np.float32).reshape(HID, HID).astype(bfnp),
        "bo_row": np.ascontiguousarray(
            np.asarray(bo, dtype=np.float32).reshape(1, HID)
        ),
    }
    in_maps = []
    for j in range(NCORES):
        m = dict(shared)
        m["wq_h"] = np.ascontiguousarray(wq3[:, j, :]).astype(bfnp)
        m["bq_col"] = np.ascontiguousarray(bq2[j].reshape(D, 1))
        in_maps.append(m)
    return in_maps


def _run(inputs, trace=False, **trace_kwargs):
    from concourse.bass_utils import run_bass_kernel_spmd

    nc = _build()
    in_maps = _make_in_maps(
        inputs["x"],
        inputs["wq"],
        inputs["bq"],
        inputs["wkv"],
        inputs["bkv"],
        inputs["wo"],
        inputs["bo"],
    )
    res = run_bass_kernel_spmd(
        nc, in_maps, list(range(NCORES)), trace=trace, **trace_kwargs
    )
    outs = [np.asarray(res.results[j]["out"]) for j in range(NCORES)]
    full = np.concatenate(outs, axis=0).reshape(1, S, HID).astype(np.float32)
    return full, res


def kernel(**inputs):
    full, _ = _run(inputs, trace=False)
    return full


if __name__ == "__main__":
    rng = np.random.default_rng(0)
    ins = {
        "x": rng.standard_normal((1, S, HID), dtype=np.float32),
        "wq": rng.standard_normal((HID, NH, D), dtype=np.float32) / 45.25,
        "bq": np.zeros((NH, D), np.float32),
        "wkv": rng.standard_normal((HID, 1, D), dtype=np.float32) / 45.25,
        "bkv": np.zeros((1, D), np.float32),
        "wo": rng.standard_normal((NH, D, HID), dtype=np.float32) / 45.25,
        "bo": np.zeros((HID,), np.float32),
        "mask": np.tril(np.ones((S, S), bool))[None, None],
    }
    out = kernel(**ins)
    print("out", out.shape, out.dtype, float(np.abs(out).max()))


# revision 13
# speedup vs baseline: 1.3463x; 1.0229x over previous
"""GQA attention kernel for Trainium2, 8 NeuronCores — v3.

Problem: B=1, S=4096, HIDDEN=2048, 8 query heads x d=256, 1 shared KV head,
causal mask, fp32 in/out.

Sharding: full tensor-parallel over heads with a replicated input.
Host-side, x is transposed/replicated to all cores in bf16 with an SBUF-
matched layout so every big load is ONE coalesced DMA (DMA dispatch on the
sync queue costs ~0.6us per instruction, so few/fat DMAs matter).
Core j owns head j end-to-end:
  1. stream xT in 4 chunks of 1024 rows; project q (own head only) and kv
     (replicated — cheaper than the AllGather it replaces) with N=1024
     bf16 matmuls; PE-transpose kv into rows-major for the AV stationary.
  2. causal flash attention for head j over all 4096 rows (fp32 PSUM, no
     max subtraction), diagonal blocks narrowed to the causal range.
  3. normalize, one bf16 AllToAll (head-major -> row-block-major).
  4. output projection of own 512-row block against full wo (N=1024).
Host concatenates the 8 row blocks.
"""

import sys

import numpy as np

sys.path.insert(0, "/opt/trn_rl_repo")

S = 4096
HID = 2048
NH = 8
D = 256
NCORES = 8
R = 512  # output rows per core / q rows per attention block
CH = 1024  # projection chunk rows
NCH = S // CH
NEG = -1.0e9
SCALE = 1.0 / 16.0  # 1/sqrt(256)

_BUILT = None


def _build():
    global _BUILT
    if _BUILT is not None:
        return _BUILT

    from contextlib import ExitStack

    import ml_dtypes

    from concourse import bacc, tile
    from concourse.bass import mybir

    dt = mybir.dt
    f32 = dt.float32
    bf16 = dt.bfloat16
    bfnp = ml_dtypes.bfloat16
    AF = mybir.ActivationFunctionType

    nc = bacc.Bacc(
        "TRN2",
        target_bir_lowering=False,
        debug=False,
        num_devices=NCORES,
    )

    # ---- DRAM I/O (host-side layouts matched to SBUF tiles) ----
    # xT_d[c, p, hs*CH+col] = x[CH*c+col, 128*hs+p]
    xT_d = nc.dram_tensor("xT", [NCH, 128, 16 * CH], bf16, kind="ExternalInput")
    # wqkv_d[p, hs*512 + j] = (wq_head | wkv)[128*hs+p, j]  (j<256 -> wq)
    wqkv_d = nc.dram_tensor("wqkv", [128, 16 * 512], bf16, kind="ExternalInput")
    # bqkv_d columns: [bq_dh0, bq_dh1, bkv_dh0, bkv_dh1]
    bqkv_d = nc.dram_tensor("bqkv", [128, 4], f32, kind="ExternalInput")
    # wo_d[p, k*HID + col] = wo2[(128*k+p), col]
    wo_d = nc.dram_tensor("wo2d", [128, 16 * HID], bf16, kind="ExternalInput")
    bo_row = nc.dram_tensor("bo_row", [1, HID], bf16, kind="ExternalInput")
    out = nc.dram_tensor("out", [R, HID], f32, kind="ExternalOutput")

    # ---- collective buffers ----
    grp = [list(range(NCORES))]
    ao_send = nc.dram_tensor("ao_send", [NH * D, R], bf16)
    ao_recv = nc.dram_tensor("ao_recv", [NH * D, R], bf16)

    # ---- compile-time constants ----
    ident_np = np.eye(128).astype(bfnp)
    ones_col_np = np.ones((128, 1)).astype(bfnp)
    ones_row_np = np.ones((1, 128)).astype(bfnp)
    # one shared diagonal mask: every diagonal (grel, sl) sub-block equals
    # m0[kappa, col - rel0] with m0[kappa, c] = NEG iff kappa > c
    kappa = np.arange(128)[:, None]
    cols = np.arange(512)[None, :]
    mask_np = np.where(kappa <= cols, 0.0, NEG).astype(np.float32)
    ident_d = nc.inline_tensor(ident_np, "ident")
    ones_col_d = nc.inline_tensor(ones_col_np, "ones_col")
    ones_row_d = nc.inline_tensor(ones_row_np, "ones_row")
    mask_d = nc.inline_tensor(mask_np, "mask_const")

    with tile.TileContext(nc) as tc:
        with ExitStack() as top:
            wo_pool = top.enter_context(tc.tile_pool(name="wo", bufs=1))
            cpool = top.enter_context(tc.tile_pool(name="const", bufs=1))

            # pools alive through projections + attention (closed before
            # phase 3 to make room for aoT/osb)
            big_stack = top.enter_context(ExitStack())
            big = big_stack.enter_context(tc.tile_pool(name="big", bufs=1))
            qT = big.tile([128, 2 * S], bf16, tag="qT")  # [d-slice, rows]
            kvT = big.tile([128, 2 * S], bf16, tag="kvT")  # [d-slice, keys]
            kv_sb = big.tile([128, 32 * D], bf16, tag="kv")  # rows-major kv
            mask_sb = big.tile([128, 512], f32, tag="mask")

            with ExitStack() as ph1:
                wpool = ph1.enter_context(tc.tile_pool(name="w", bufs=1))
                xr_pool = ph1.enter_context(tc.tile_pool(name="xr", bufs=2))

                # critical-path loads first: weights, then x chunks 0/1
                wqkv_sb = wpool.tile([128, 16 * 512], bf16, tag="wqkv")
                nc.sync.dma_start(wqkv_sb[:], wqkv_d[:])
                xrs = []
                for c in range(2):
                    xr = xr_pool.tile([128, 16 * CH], bf16, tag="xr", name=f"xr{c}")
                    nc.sync.dma_start(xr[:], xT_d[c])
                    xrs.append(xr)

                # remaining constants
                ident = cpool.tile([128, 128], bf16, tag="ident")
                nc.sync.dma_start(ident[:], ident_d[:])
                ones_col = cpool.tile([128, 1], bf16, tag="ones_col")
                nc.sync.dma_start(ones_col[:], ones_col_d[:])
                ones_row = cpool.tile([1, 128], bf16, tag="ones_row")
                nc.sync.dma_start(ones_row[:], ones_row_d[:])
                bqkv_sb = cpool.tile([128, 4], f32, tag="bqkv")
                nc.sync.dma_start(bqkv_sb[:], bqkv_d[:])
                nc.sync.dma_start(mask_sb[:], mask_d[:])
                bor_sb = cpool.tile([1, HID], bf16, tag="bor")
                nc.sync.dma_start(bor_sb[:], bo_row[:])

                # ============ phase 1: projections (4 chunks of 1024) ========
                pj_psum = ph1.enter_context(
                    tc.tile_pool(name="pj_psum", bufs=6, space="PSUM")
                )
                tp_psum = ph1.enter_context(
                    tc.tile_pool(name="tp_psum", bufs=2, space="PSUM")
                )
                for c in range(NCH):
                    if c < 2:
                        xr = xrs[c]
                    else:
                        xr = xr_pool.tile(
                            [128, 16 * CH], bf16, tag="xr", name=f"xr{c}"
                        )
                        nc.sync.dma_start(xr[:], xT_d[c])
                    for dh in range(2):
                        for wofs, dst, bcol in (
                            (0, qT, 0),
                            (256, kvT, 2),
                        ):
                            for half in range(CH // 512):
                                pp = pj_psum.tile([128, 512], f32, tag="pj")
                                for hs in range(16):
                                    nc.tensor.matmul(
                                        pp[:],
                                        wqkv_sb[
                                            :,
                                            512 * hs + wofs + 128 * dh : 512 * hs
                                            + wofs
                                            + 128 * dh
                                            + 128,
                                        ],
                                        xr[
                                            :,
                                            CH * hs + 512 * half : CH * hs
                                            + 512 * half
                                            + 512,
                                        ],
                                        start=(hs == 0),
                                        stop=(hs == 15),
                                    )
                                nc.scalar.activation(
                                    dst[
                                        :,
                                        S * dh + CH * c + 512 * half : S * dh
                                        + CH * c
                                        + 512 * half
                                        + 512,
                                    ],
                                    pp[:],
                                    AF.Identity,
                                    bias=bqkv_sb[:, bcol + dh : bcol + dh + 1],
                                )
                    # rows-major kv via PE transpose (bf16)
                    for i4 in range(CH // 128):
                        kt = (CH // 128) * c + i4
                        for dh in range(2):
                            tp = tp_psum.tile([128, 128], bf16, tag="tp")
                            nc.tensor.transpose(
                                tp[:],
                                kvT[
                                    :,
                                    S * dh + CH * c + 128 * i4 : S * dh
                                    + CH * c
                                    + 128 * i4
                                    + 128,
                                ],
                                ident[:],
                            )
                            nc.vector.tensor_copy(
                                kv_sb[:, D * kt + 128 * dh : D * kt + 128 * dh + 128],
                                tp[:],
                            )

            # prefetch wo during attention (one coalesced DMA)
            wo_sb = wo_pool.tile([128, 16 * HID], bf16, tag="wo")
            nc.sync.dma_start(wo_sb[:], wo_d[:])

            # ============ phase 2: causal flash attention ============
            with ExitStack() as ph2:
                s_psum = ph2.enter_context(
                    tc.tile_pool(name="s_psum", bufs=2, space="PSUM")
                )
                ao_psum = ph2.enter_context(
                    tc.tile_pool(name="ao_psum", bufs=3, space="PSUM")
                )
                den_psum = ph2.enter_context(
                    tc.tile_pool(name="den_psum", bufs=1, space="PSUM")
                )
                p_pool = ph2.enter_context(tc.tile_pool(name="p", bufs=3))
                nrm_pool = ph2.enter_context(tc.tile_pool(name="nrm", bufs=2))
                aon_pool = ph2.enter_context(tc.tile_pool(name="aon", bufs=4))

                for b in range(7, -1, -1):
                    aops = [
                        ao_psum.tile([128, R], f32, tag="aops", name=f"aops{b}_{i}")
                        for i in range(2)
                    ]
                    denp = den_psum.tile([1, R], f32, tag="denp")
                    ngroups = 2 * (b + 1)
                    for kg in range(ngroups):
                        diag = kg >= 2 * b
                        grel = kg - 2 * b
                        st = s_psum.tile([128, 1024], f32, tag="st")
                        rel = [
                            256 * grel + 128 * sl if diag else 0 for sl in range(2)
                        ]
                        for sl in range(2):
                            k = 2 * kg + sl
                            r0 = rel[sl]
                            for dh in range(2):
                                nc.tensor.matmul(
                                    st[:, 512 * sl + r0 : 512 * sl + 512],
                                    kvT[:, S * dh + 128 * k : S * dh + 128 * k + 128],
                                    qT[:, S * dh + R * b + r0 : S * dh + R * b + R],
                                    start=(dh == 0),
                                    stop=(dh == 1),
                                )
                        if diag:
                            for sl in range(2):
                                r0 = rel[sl]
                                nc.vector.tensor_add(
                                    st[:, 512 * sl + r0 : 512 * sl + 512],
                                    st[:, 512 * sl + r0 : 512 * sl + 512],
                                    mask_sb[:, 0 : 512 - r0],
                                )
                        pt = p_pool.tile([128, 1024], bf16, tag="pt")
                        for sl in range(2):
                            r0 = rel[sl]
                            nc.scalar.activation(
                                pt[:, 512 * sl + r0 : 512 * sl + 512],
                                st[:, 512 * sl + r0 : 512 * sl + 512],
                                AF.Exp,
                                scale=SCALE,
                            )
                        for sl in range(2):
                            k = 2 * kg + sl
                            r0 = rel[sl]
                            first = kg == 0 and sl == 0
                            last = kg == ngroups - 1 and sl == 1
                            nc.tensor.matmul(
                                denp[:, r0:R],
                                ones_col[:],
                                pt[:, 512 * sl + r0 : 512 * sl + 512],
                                start=first,
                                stop=last,
                            )
                            for dh in range(2):
                                nc.tensor.matmul(
                                    aops[dh][:, r0:R],
                                    kv_sb[:, D * k + 128 * dh : D * k + 128 * dh + 128],
                                    pt[:, 512 * sl + r0 : 512 * sl + 512],
                                    start=first,
                                    stop=last,
                                )
                    # normalize + send (bf16)
                    den_sb = nrm_pool.tile([1, R], f32, tag="den_sb")
                    nc.vector.reciprocal(den_sb[:], denp[:])
                    bc = nrm_pool.tile([128, R], f32, tag="bc")
                    nc.gpsimd.partition_broadcast(bc[:], den_sb[:])
                    for dh in range(2):
                        aon = aon_pool.tile([128, R], bf16, tag="aon")
                        nc.vector.tensor_mul(aon[:], aops[dh][:], bc[:])
                        nc.sync.dma_start(
                            ao_send[D * b + 128 * dh : D * b + 128 * dh + 128, :],
                            aon[:],
                        )
                nc.gpsimd.collective_compute(
                    "AllToAll",
                    mybir.AluOpType.bypass,
                    replica_groups=grp,
                    ins=[ao_send[:]],
                    outs=[ao_recv[:]],
                )

            big_stack.close()  # free qT/kvT/kv/mask before phase 3

            # ============ phase 3: output projection ============
            with ExitStack() as ph3:
                o_in = ph3.enter_context(tc.tile_pool(name="o_in", bufs=1))
                aoT = o_in.tile([128, 16 * R], bf16, tag="aoT")
                for k in range(16):
                    nc.sync.dma_start(
                        aoT[:, R * k : R * k + R],
                        ao_recv[128 * k : 128 * k + 128, :],
                    )
                o_psum = ph3.enter_context(
                    tc.tile_pool(name="o_psum", bufs=4, space="PSUM")
                )
                o_out = ph3.enter_context(tc.tile_pool(name="o_out", bufs=2))
                for rc in range(4):
                    osb = o_out.tile([128, HID], f32, tag="osb")
                    for ncol in range(4):
                        ps = o_psum.tile([128, 512], f32, tag="ops")
                        for k in range(16):
                            nc.tensor.matmul(
                                ps[:],
                                aoT[:, R * k + 128 * rc : R * k + 128 * rc + 128],
                                wo_sb[
                                    :,
                                    HID * k + 512 * ncol : HID * k + 512 * ncol + 512,
                                ],
                                start=(k == 0),
                                stop=False,
                            )
                        nc.tensor.matmul(
                            ps[:],
                            ones_row[:],
                            bor_sb[:, 512 * ncol : 512 * ncol + 512],
                            start=False,
                            stop=True,
                        )
                        nc.vector.tensor_copy(
                            osb[:, 512 * ncol : 512 * ncol + 512], ps[:]
                        )
                    nc.sync.dma_start(out[128 * rc : 128 * rc + 128, :], osb[:])

    nc.compile()
    _BUILT = nc
    return nc


def _make_in_maps(x, wq, bq, wkv, bkv, wo, bo):
    import ml_dtypes

    bfnp = ml_dtypes.bfloat16
    x2d = np.asarray(x, dtype=np.float32).reshape(S, HID)
    # xT_d[c, p, hs*CH+col] = x[CH*c+col, 128*hs+p]
    xT = (
        x2d.reshape(NCH, CH, 16, 128)
        .transpose(0, 3, 2, 1)
        .reshape(NCH, 128, 16 * CH)
        .astype(bfnp)
    )
    wq3 = np.asarray(wq, dtype=np.float32).reshape(HID, NH, D)
    bq2 = np.asarray(bq, dtype=np.float32).reshape(NH, D)
    bkv1 = np.asarray(bkv, dtype=np.float32).reshape(D)
    wkv2 = np.asarray(wkv, dtype=np.float32).reshape(HID, D)
    wo2 = np.asarray(wo, dtype=np.float32).reshape(HID, HID)
    wo_h = (
        wo2.reshape(16, 128, HID).transpose(1, 0, 2).reshape(128, 16 * HID).astype(bfnp)
    )
    shared = {
        "xT": xT,
        "wo2d": wo_h,
        "bo_row": np.asarray(bo, dtype=np.float32).reshape(1, HID).astype(bfnp),
    }
    in_maps = []
    for j in range(NCORES):
        m = dict(shared)
        wq_h = wq3[:, j, :]  # [HID, D]
        qk = np.concatenate(
            [wq_h.reshape(16, 128, D), wkv2.reshape(16, 128, D)], axis=2
        )  # [16, 128, 512]
        m["wqkv"] = qk.transpose(1, 0, 2).reshape(128, 16 * 512).astype(bfnp)
        bq_h = bq2[j]
        bqkv = np.stack(
            [bq_h[:128], bq_h[128:], bkv1[:128], bkv1[128:]], axis=1
        )  # [128, 4]
        m["bqkv"] = np.ascontiguousarray(bqkv.astype(np.float32))
        in_maps.append(m)
    return in_maps


def _run(inputs, trace=False, **trace_kwargs):
    from concourse.bass_utils import run_bass_kernel_spmd

    nc = _build()
    in_maps = _make_in_maps(
        inputs["x"],
        inputs["wq"],
        inputs["bq"],
        inputs["wkv"],
        inputs["bkv"],
        inputs["wo"],
        inputs["bo"],
    )
    res = run_bass_kernel_spmd(
        nc, in_maps, list(range(NCORES)), trace=trace, **trace_kwargs
    )
    outs = [np.asarray(res.results[j]["out"]) for j in range(NCORES)]
    full = np.concatenate(outs, axis=0).reshape(1, S, HID).astype(np.float32)
    return full, res


def kernel(**inputs):
    full, _ = _run(inputs, trace=False)
    return full


if __name__ == "__main__":
    rng = np.random.default_rng(0)
    ins = {
        "x": rng.standard_normal((1, S, HID), dtype=np.float32),
        "wq": rng.standard_normal((HID, NH, D), dtype=np.float32) / 45.25,
        "bq": np.zeros((NH, D), np.float32),
        "wkv": rng.standard_normal((HID, 1, D), dtype=np.float32) / 45.25,
        "bkv": np.zeros((1, D), np.float32),
        "wo": rng.standard_normal((NH, D, HID), dtype=np.float32) / 45.25,
        "bo": np.zeros((HID,), np.float32),
        "mask": np.tril(np.ones((S, S), bool))[None, None],
    }
    out = kernel(**ins)
    print("out", out.shape, out.dtype, float(np.abs(out).max()))


# revision 27
# speedup vs baseline: 1.3635x; 1.0127x over previous
"""GQA attention kernel for Trainium2, 8 NeuronCores — v3.

Problem: B=1, S=4096, HIDDEN=2048, 8 query heads x d=256, 1 shared KV head,
causal mask, fp32 in/out.

Sharding: full tensor-parallel over heads with a replicated input.
Host-side, x is transposed/replicated to all cores in bf16 with an SBUF-
matched layout so every big load is ONE coalesced DMA (DMA dispatch on the
sync queue costs ~0.6us per instruction, so few/fat DMAs matter).
Core j owns head j end-to-end:
  1. stream xT in 4 chunks of 1024 rows; project q (own head only) and kv
     (replicated — cheaper than the AllGather it replaces) with N=1024
     bf16 matmuls; PE-transpose kv into rows-major for the AV stationary.
  2. causal flash attention for head j over all 4096 rows (fp32 PSUM, no
     max subtraction), diagonal blocks narrowed to the causal range.
  3. normalize, one bf16 AllToAll (head-major -> row-block-major).
  4. output projection of own 512-row block against full wo (N=1024).
Host concatenates the 8 row blocks.
"""

import sys

import numpy as np

sys.path.insert(0, "/opt/trn_rl_repo")

S = 4096
HID = 2048
NH = 8
D = 256
NCORES = 8
R = 512  # output rows per core / q rows per attention block
CH = 1024  # projection chunk rows
NCH = S // CH
NEG = -1.0e9
SCALE = 1.0 / 16.0  # 1/sqrt(256)

_BUILT = None


def _build():
    global _BUILT
    if _BUILT is not None:
        return _BUILT

    from contextlib import ExitStack

    import ml_dtypes

    from concourse import bacc, tile
    from concourse.bass import mybir

    dt = mybir.dt
    f32 = dt.float32
    bf16 = dt.bfloat16
    bfnp = ml_dtypes.bfloat16
    AF = mybir.ActivationFunctionType

    nc = bacc.Bacc(
        "TRN2",
        target_bir_lowering=False,
        debug=False,
        num_devices=NCORES,
    )

    # ---- DRAM I/O (host-side layouts matched to SBUF tiles) ----
    # xT_d[c, p, hs*CH+col] = x[CH*c+col, 128*hs+p]
    xT_d = nc.dram_tensor("xT", [NCH, 128, 16 * CH], bf16, kind="ExternalInput")
    # wqkv_d[p, hs*512 + j] = (wq_head | wkv)[128*hs+p, j]  (j<256 -> wq)
    wqkv_d = nc.dram_tensor("wqkv", [128, 16 * 512], bf16, kind="ExternalInput")
    # bqkv_d columns: [bq_dh0, bq_dh1, bkv_dh0, bkv_dh1]
    bqkv_d = nc.dram_tensor("bqkv", [128, 4], f32, kind="ExternalInput")
    # wo_d[p, k*HID + col] = wo2[(128*k+p), col]
    wo_d = nc.dram_tensor("wo2d", [128, 16 * HID], bf16, kind="ExternalInput")
    bo_row = nc.dram_tensor("bo_row", [1, HID], bf16, kind="ExternalInput")
    out = nc.dram_tensor("out", [R, HID], f32, kind="ExternalOutput")

    # ---- collective buffers ----
    grp = [list(range(NCORES))]
    ao_send = nc.dram_tensor("ao_send", [NH * D, R], bf16)
    ao_recv = nc.dram_tensor("ao_recv", [NH * D, R], bf16)

    # ---- compile-time constants ----
    fp8 = dt.float8e4
    fp8np = ml_dtypes.float8_e4m3fn
    ident_np = np.eye(128).astype(bfnp)
    ones_col_np = np.ones((128, 1)).astype(bfnp)
    ones_row_np = np.ones((1, 128)).astype(bfnp)
    ones8_np = np.ones((128, 2, 32), dtype=fp8np)
    # one shared diagonal mask: every diagonal (grel, sl) sub-block equals
    # m0[kappa, col - rel0] with m0[kappa, c] = NEG iff kappa > c
    kappa = np.arange(128)[:, None]
    cols = np.arange(512)[None, :]
    mask_np = np.where(kappa <= cols, 0.0, NEG).astype(np.float32)
    ident_d = nc.inline_tensor(ident_np, "ident")
    ones_col_d = nc.inline_tensor(ones_col_np, "ones_col")
    ones_row_d = nc.inline_tensor(ones_row_np, "ones_row")
    ones8_d = nc.inline_tensor(ones8_np, "ones8")
    mask_d = nc.inline_tensor(mask_np, "mask_const")

    with tile.TileContext(nc) as tc:
        with ExitStack() as top:
            wo_pool = top.enter_context(tc.tile_pool(name="wo", bufs=1))
            cpool = top.enter_context(tc.tile_pool(name="const", bufs=1))

            # pools alive through projections + attention (closed before
            # phase 3 to make room for aoT/osb)
            big_stack = top.enter_context(ExitStack())
            big = big_stack.enter_context(tc.tile_pool(name="big", bufs=1))
            qT = big.tile([128, 2 * S], bf16, tag="qT")  # [d-slice, rows]
            kvT = big.tile([128, 2 * S], bf16, tag="kvT")  # [d-slice, keys]
            kv_sb = big.tile([128, 32 * D], bf16, tag="kv")  # rows-major kv
            mask_sb = big.tile([128, 512], f32, tag="mask")

            with ExitStack() as ph1:
                wpool = ph1.enter_context(tc.tile_pool(name="w", bufs=1))
                xr_pool = ph1.enter_context(tc.tile_pool(name="xr", bufs=2))

                # critical-path loads first: weights, then x chunks 0/1
                wqkv_sb = wpool.tile([128, 16 * 512], bf16, tag="wqkv")
                nc.sync.dma_start(wqkv_sb[:], wqkv_d[:])
                xrs = []
                for c in range(2):
                    xr = xr_pool.tile([128, 16 * CH], bf16, tag="xr", name=f"xr{c}")
                    nc.sync.dma_start(xr[:], xT_d[c])
                    xrs.append(xr)

                # remaining constants
                ident = cpool.tile([128, 128], bf16, tag="ident")
                nc.sync.dma_start(ident[:], ident_d[:])
                ones_col = cpool.tile([128, 1], bf16, tag="ones_col")
                nc.sync.dma_start(ones_col[:], ones_col_d[:])
                ones_row = cpool.tile([1, 128], bf16, tag="ones_row")
                nc.sync.dma_start(ones_row[:], ones_row_d[:])
                ones8 = cpool.tile([128, 2, 32], fp8, tag="ones8")
                nc.sync.dma_start(ones8[:], ones8_d[:])
                bqkv_sb = cpool.tile([128, 4], f32, tag="bqkv")
                nc.sync.dma_start(bqkv_sb[:], bqkv_d[:])
                nc.sync.dma_start(mask_sb[:], mask_d[:])
                bor_sb = cpool.tile([1, HID], bf16, tag="bor")
                nc.sync.dma_start(bor_sb[:], bo_row[:])

                # ============ phase 1: projections (4 chunks of 1024) ========
                pj_psum = ph1.enter_context(
                    tc.tile_pool(name="pj_psum", bufs=6, space="PSUM")
                )
                tp_psum = ph1.enter_context(
                    tc.tile_pool(name="tp_psum", bufs=2, space="PSUM")
                )
                for c in range(NCH):
                    if c < 2:
                        xr = xrs[c]
                    else:
                        xr = xr_pool.tile(
                            [128, 16 * CH], bf16, tag="xr", name=f"xr{c}"
                        )
                        nc.sync.dma_start(xr[:], xT_d[c])
                    for dh in range(2):
                        for wofs, dst, bcol in (
                            (0, qT, 0),
                            (256, kvT, 2),
                        ):
                            for half in range(CH // 512):
                                pp = pj_psum.tile([128, 512], f32, tag="pj")
                                for hs in range(16):
                                    nc.tensor.matmul(
                                        pp[:],
                                        wqkv_sb[
                                            :,
                                            512 * hs + wofs + 128 * dh : 512 * hs
                                            + wofs
                                            + 128 * dh
                                            + 128,
                                        ],
                                        xr[
                                            :,
                                            CH * hs + 512 * half : CH * hs
                                            + 512 * half
                                            + 512,
                                        ],
                                        start=(hs == 0),
                                        stop=(hs == 15),
                                    )
                                nc.scalar.activation(
                                    dst[
                                        :,
                                        S * dh + CH * c + 512 * half : S * dh
                                        + CH * c
                                        + 512 * half
                                        + 512,
                                    ],
                                    pp[:],
                                    AF.Identity,
                                    bias=bqkv_sb[:, bcol + dh : bcol + dh + 1],
                                )
                    # rows-major kv via PE transpose (bf16)
                    for i4 in range(CH // 128):
                        kt = (CH // 128) * c + i4
                        for dh in range(2):
                            tp = tp_psum.tile([128, 128], bf16, tag="tp")
                            nc.tensor.transpose(
                                tp[:],
                                kvT[
                                    :,
                                    S * dh + CH * c + 128 * i4 : S * dh
                                    + CH * c
                                    + 128 * i4
                                    + 128,
                                ],
                                ident[:],
                            )
                            nc.vector.tensor_copy(
                                kv_sb[:, D * kt + 128 * dh : D * kt + 128 * dh + 128],
                                tp[:],
                            )

            # prefetch wo during attention (one coalesced DMA)
            wo_sb = wo_pool.tile([128, 16 * HID], bf16, tag="wo")
            nc.sync.dma_start(wo_sb[:], wo_d[:])

            # ============ phase 2: causal flash attention ============
            with ExitStack() as ph2:
                s_psum = ph2.enter_context(
                    tc.tile_pool(name="s_psum", bufs=2, space="PSUM")
                )
                ao_psum = ph2.enter_context(
                    tc.tile_pool(name="ao_psum", bufs=3, space="PSUM")
                )
                den_psum = ph2.enter_context(
                    tc.tile_pool(name="den_psum", bufs=1, space="PSUM")
                )
                p_pool = ph2.enter_context(tc.tile_pool(name="p", bufs=3))
                p8_pool = ph2.enter_context(tc.tile_pool(name="p8", bufs=3))
                nrm_pool = ph2.enter_context(tc.tile_pool(name="nrm", bufs=2))
                aon_pool = ph2.enter_context(tc.tile_pool(name="aon", bufs=4))

                for b in range(7, -1, -1):
                    aops = [
                        ao_psum.tile([128, R], f32, tag="aops", name=f"aops{b}_{i}")
                        for i in range(2)
                    ]
                    denp = den_psum.tile([32, R], f32, tag="denp")
                    ngroups = 2 * (b + 1)
                    for kg in range(ngroups):
                        diag = kg >= 2 * b
                        grel = kg - 2 * b
                        st = s_psum.tile([128, 1024], f32, tag="st")
                        rel = [
                            256 * grel + 128 * sl if diag else 0 for sl in range(2)
                        ]
                        for sl in range(2):
                            k = 2 * kg + sl
                            r0 = rel[sl]
                            for dh in range(2):
                                nc.tensor.matmul(
                                    st[:, 512 * sl + r0 : 512 * sl + 512],
                                    kvT[:, S * dh + 128 * k : S * dh + 128 * k + 128],
                                    qT[:, S * dh + R * b + r0 : S * dh + R * b + R],
                                    start=(dh == 0),
                                    stop=(dh == 1),
                                )
                        if diag:
                            for sl in range(2):
                                r0 = rel[sl]
                                nc.vector.tensor_add(
                                    st[:, 512 * sl + r0 : 512 * sl + 512],
                                    st[:, 512 * sl + r0 : 512 * sl + 512],
                                    mask_sb[:, 0 : 512 - r0],
                                )
                        pt = p_pool.tile([128, 1024], bf16, tag="pt")
                        for sl in range(2):
                            r0 = rel[sl]
                            nc.scalar.activation(
                                pt[:, 512 * sl + r0 : 512 * sl + 512],
                                st[:, 512 * sl + r0 : 512 * sl + 512],
                                AF.Exp,
                                scale=SCALE,
                            )
                        if not diag:
                            # denominator via one fp8 DoubleRow matmul over
                            # both slices (den is a coherent positive sum —
                            # fp8 rounding averages out)
                            pt8 = p8_pool.tile([128, 2, 512], fp8, tag="pt8")
                            for sl in range(2):
                                nc.scalar.activation(
                                    pt8[:, sl, :],
                                    st[:, 512 * sl : 512 * sl + 512],
                                    AF.Exp,
                                    scale=SCALE,
                                )
                            nc.tensor.matmul(
                                denp[:, 0:R],
                                ones8[:],
                                pt8[:],
                                start=(kg == 0),
                                stop=False,
                                perf_mode=mybir.MatmulPerfMode.DoubleRow,
                                skip_group_check=True,
                            )
                        for sl in range(2):
                            k = 2 * kg + sl
                            r0 = rel[sl]
                            first = kg == 0 and sl == 0
                            last = kg == ngroups - 1 and sl == 1
                            if diag:
                                nc.tensor.matmul(
                                    denp[0:1, r0:R],
                                    ones_col[:],
                                    pt[:, 512 * sl + r0 : 512 * sl + 512],
                                    start=first,
                                    stop=last,
                                    skip_group_check=True,
                                )
                            for dh in range(2):
                                nc.tensor.matmul(
                                    aops[dh][:, r0:R],
                                    kv_sb[:, D * k + 128 * dh : D * k + 128 * dh + 128],
                                    pt[:, 512 * sl + r0 : 512 * sl + 512],
                                    start=first,
                                    stop=last,
                                )
                    # normalize + send (bf16)
                    den_sb = nrm_pool.tile([1, R], f32, tag="den_sb")
                    nc.vector.reciprocal(den_sb[:], denp[0:1, :])
                    bc = nrm_pool.tile([128, R], f32, tag="bc")
                    nc.gpsimd.partition_broadcast(bc[:], den_sb[:])
                    for dh in range(2):
                        aon = aon_pool.tile([128, R], bf16, tag="aon")
                        nc.vector.tensor_mul(aon[:], aops[dh][:], bc[:])
                        nc.sync.dma_start(
                            ao_send[D * b + 128 * dh : D * b + 128 * dh + 128, :],
                            aon[:],
                        )
                nc.gpsimd.collective_compute(
                    "AllToAll",
                    mybir.AluOpType.bypass,
                    replica_groups=grp,
                    ins=[ao_send[:]],
                    outs=[ao_recv[:]],
                )

            big_stack.close()  # free qT/kvT/kv/mask before phase 3

            # ============ phase 3: output projection ============
            with ExitStack() as ph3:
                o_in = ph3.enter_context(tc.tile_pool(name="o_in", bufs=1))
                aoT = o_in.tile([128, 16 * R], bf16, tag="aoT")
                for k in range(16):
                    nc.sync.dma_start(
                        aoT[:, R * k : R * k + R],
                        ao_recv[128 * k : 128 * k + 128, :],
                    )
                o_psum = ph3.enter_context(
                    tc.tile_pool(name="o_psum", bufs=4, space="PSUM")
                )
                o_out = ph3.enter_context(tc.tile_pool(name="o_out", bufs=2))
                for rc in range(4):
                    osb = o_out.tile([128, HID], f32, tag="osb")
                    for ncol in range(4):
                        ps = o_psum.tile([128, 512], f32, tag="ops")
                        for k in range(16):
                            nc.tensor.matmul(
                                ps[:],
                                aoT[:, R * k + 128 * rc : R * k + 128 * rc + 128],
                                wo_sb[
                                    :,
                                    HID * k + 512 * ncol : HID * k + 512 * ncol + 512,
                                ],
                                start=(k == 0),
                                stop=False,
                            )
                        nc.tensor.matmul(
                            ps[:],
                            ones_row[:],
                            bor_sb[:, 512 * ncol : 512 * ncol + 512],
                            start=False,
                            stop=True,
                        )
                        nc.vector.tensor_copy(
                            osb[:, 512 * ncol : 512 * ncol + 512], ps[:]
                        )
                    nc.sync.dma_start(out[128 * rc : 128 * rc + 128, :], osb[:])

    nc.compile()
    _BUILT = nc
    return nc


def _make_in_maps(x, wq, bq, wkv, bkv, wo, bo):
    import ml_dtypes

    bfnp = ml_dtypes.bfloat16
    x2d = np.asarray(x, dtype=np.float32).reshape(S, HID)
    # xT_d[c, p, hs*CH+col] = x[CH*c+col, 128*hs+p]
    xT = (
        x2d.reshape(NCH, CH, 16, 128)
        .transpose(0, 3, 2, 1)
        .reshape(NCH, 128, 16 * CH)
        .astype(bfnp)
    )
    wq3 = np.asarray(wq, dtype=np.float32).reshape(HID, NH, D)
    bq2 = np.asarray(bq, dtype=np.float32).reshape(NH, D)
    bkv1 = np.asarray(bkv, dtype=np.float32).reshape(D)
    wkv2 = np.asarray(wkv, dtype=np.float32).reshape(HID, D)
    wo2 = np.asarray(wo, dtype=np.float32).reshape(HID, HID)
    wo_h = (
        wo2.reshape(16, 128, HID).transpose(1, 0, 2).reshape(128, 16 * HID).astype(bfnp)
    )
    shared = {
        "xT": xT,
        "wo2d": wo_h,
        "bo_row": np.asarray(bo, dtype=np.float32).reshape(1, HID).astype(bfnp),
    }
    in_maps = []
    for j in range(NCORES):
        m = dict(shared)
        wq_h = wq3[:, j, :]  # [HID, D]
        qk = np.concatenate(
            [wq_h.reshape(16, 128, D), wkv2.reshape(16, 128, D)], axis=2
        )  # [16, 128, 512]
        m["wqkv"] = qk.transpose(1, 0, 2).reshape(128, 16 * 512).astype(bfnp)
        bq_h = bq2[j]
        bqkv = np.stack(
            [bq_h[:128], bq_h[128:], bkv1[:128], bkv1[128:]], axis=1
        )  # [128, 4]
        m["bqkv"] = np.ascontiguousarray(bqkv.astype(np.float32))
        in_maps.append(m)
    return in_maps


def _run(inputs, trace=False, **trace_kwargs):
    from concourse.bass_utils import run_bass_kernel_spmd

    nc = _build()
    in_maps = _make_in_maps(
        inputs["x"],
        inputs["wq"],
        inputs["bq"],
        inputs["wkv"],
        inputs["bkv"],
        inputs["wo"],
        inputs["bo"],
    )
    res = run_bass_kernel_spmd(
        nc, in_maps, list(range(NCORES)), trace=trace, **trace_kwargs
    )
    outs = [np.asarray(res.results[j]["out"]) for j in range(NCORES)]
    full = np.concatenate(outs, axis=0).reshape(1, S, HID).astype(np.float32)
    return full, res


def kernel(**inputs):
    full, _ = _run(inputs, trace=False)
    return full


if __name__ == "__main__":
    rng = np.random.default_rng(0)
    ins = {
        "x": rng.standard_normal((1, S, HID), dtype=np.float32),
        "wq": rng.standard_normal((HID, NH, D), dtype=np.float32) / 45.25,
        "bq": np.zeros((NH, D), np.float32),
        "wkv": rng.standard_normal((HID, 1, D), dtype=np.float32) / 45.25,
        "bkv": np.zeros((1, D), np.float32),
        "wo": rng.standard_normal((NH, D, HID), dtype=np.float32) / 45.25,
        "bo": np.zeros((HID,), np.float32),
        "mask": np.tril(np.ones((S, S), bool))[None, None],
    }
    out = kernel(**ins)
    print("out", out.shape, out.dtype, float(np.abs(out).max()))
